# revision 23
# baseline (speedup 1.0000x reference)
"""Multi-head attention Trainium2 kernel (v2).

Problem: B=2, S=4096, D=512, H=8 heads, dk=dv=64 (fp32).
Sharding: head-parallel -- core c computes head c for both batches.

Per-core algorithm (head h):
  Phase A: stream x in 512-token groups; PE-transpose to x^T (bf16
    identity, fp32r data); packed [wq|wk] projection (one PSUM pass for
    Q^T and K^T, rows 0:63 / 64:127), separate V pass; drains split
    across DVE and ACT so no engine stalls.
  Phase B (qg = 256 queries x both batches per group-column):
    score groups of GRP=3 key blocks: psum tile [128, 3*512] where each
    512-col unit = [b0 256q | b1 256q]; QK row-packed across batches
    (tile_position (0,0)/(64,0), concurrent); exp split: ACT exp on the
    first SPLIT_COLS columns, DVE one-op Schraudolph (int32(s*A+B)
    bitcast as fp32) on the rest; PV: [V|1] stationary [128,65] fp32r,
    moving P [128,256], accumulated per batch into pv psum [65, 512]
    (b0 cols 0:256, b1 256:512) over all 32 key blocks.
  Epilogue per qg: PE-transpose [65,128] blocks to natural layout,
    single strided reciprocal of the 4 denominator columns, normalize
    (DVE), bias add (gpsimd), DMA out natural [128,64] blocks.
"""

import sys

sys.path.insert(0, "/opt/trn_rl_repo")

import numpy as np
import ml_dtypes

import concourse.bass as bass
import concourse.tile as tile
from concourse import bacc, mybir
from concourse.bass_utils import run_bass_kernel_spmd

FP32 = mybir.dt.float32
FP32R = mybir.dt.float32r
BF16 = mybir.dt.bfloat16
INT32 = mybir.dt.int32

B = 2
S = 4096
D = 512
DK = 64
HEADS = 8
N_CORES = 8

TG = 512          # tokens per phase-A group
QG = 256          # queries per phase-B group (per batch)
KB = 128          # keys per block
GRP = 3           # key blocks per score-psum group (3 banks)

# Schraudolph fast-exp constants: exp(s/8) ~ bitcast(int32(s*SCH_A+SCH_B))
SCH_A = float((2.0 ** 23) / np.log(2.0) / 8.0)
SCH_B = float(127.0 * 2.0 ** 23 - 487500.0)
# Columns per 1536-col score group handled by ACT (rest by the DVE+Pool
# Schraudolph pipeline: DVE mult-add to int32 scratch, Pool rounds to fp32r).
import os
EXP_MODE = os.environ.get("MHA_EXP", "dve2")  # act | dve2 | dvepool
NO_GPSIMD = os.environ.get("MHA_NOGP", "1") == "1"
CUT = int(os.environ.get("MHA_CUT", "4"))  # 1=QK 2=+exp 3=+PV 4=full


def _gp(nc):
    return nc.vector if NO_GPSIMD else nc.gpsimd
if EXP_MODE == "act":
    SPLIT3, SPLIT2 = 1536, 1024
elif EXP_MODE == "dve2":
    SPLIT3, SPLIT2 = 1056, 704
else:
    SPLIT3, SPLIT2 = 960, 640


def build_nc(s=S, reps=1, phases="AB"):
    """Build the per-core Bass program (SPMD, same NEFF on all cores)."""
    toks = B * s             # total tokens
    n_tg = toks // TG        # phase-A groups
    n_qg = s // QG           # phase-B query groups (each spans both batches)
    n_kb = s // KB           # key blocks per batch

    nc = bacc.Bacc("TRN2", target_bir_lowering=False, debug=False,
                   num_devices=N_CORES)

    x_d = nc.dram_tensor("x", [toks, D], FP32, kind="ExternalInput")
    wq_d = nc.dram_tensor("wq", [D, DK], FP32, kind="ExternalInput")
    wk_d = nc.dram_tensor("wk", [D, DK], FP32, kind="ExternalInput")
    wv_d = nc.dram_tensor("wv", [D, DK], FP32, kind="ExternalInput")
    bqk_d = nc.dram_tensor("bqk", [128, 1], FP32, kind="ExternalInput")
    bv128_d = nc.dram_tensor("bv128", [128, DK], FP32, kind="ExternalInput")
    id_d = nc.dram_tensor("identr", [128, 128], FP32, kind="ExternalInput")
    out_d = nc.dram_tensor("out", [toks, DK], FP32, kind="ExternalOutput")

    with tile.TileContext(nc) as tc:
        with tc.tile_pool(name="persist", bufs=1) as pp:
            ident = pp.tile([128, 128], FP32, tag="ident")
            nc.sync.dma_start(ident[:], id_d[:])
            # packed [wq | wk] stationary: per 128-row chunk c of D,
            # cols 0:64 = wq chunk, 64:128 = wk chunk
            wqk_n = pp.tile([128, 512], FP32, tag="wqk_n")
            wqk_v = wqk_n[:].rearrange("p (c k m) -> p c k m", k=2, m=64)
            nc.sync.dma_start(wqk_v[:, :, 0, :],
                              wq_d.rearrange("(c p) m -> p c m", p=128))
            nc.sync.dma_start(wqk_v[:, :, 1, :],
                              wk_d.rearrange("(c p) m -> p c m", p=128))
            wv_n = pp.tile([128, 256], FP32, tag="wv_n")
            nc.sync.dma_start(
                wv_n[:].rearrange("p (c m) -> p c m", m=64),
                wv_d.rearrange("(c p) m -> p c m", p=128))
            wqk_r = pp.tile([128, 512], FP32R, tag="wqk_r")
            _gp(nc).tensor_copy(wqk_r[:], wqk_n[:])
            wv_r = pp.tile([128, 256], FP32R, tag="wv_r")
            _gp(nc).tensor_copy(wv_r[:], wv_n[:])
            bqk_t = pp.tile([128, 1], FP32, tag="bqk")
            nc.sync.dma_start(bqk_t[:], bqk_d[:])
            bv128_t = pp.tile([128, DK], FP32, tag="bv128")
            nc.sync.dma_start(bv128_t[:], bv128_d[:])

            # Q^T | K^T, both batches: rows 0:64 = batch 0, 64:128 = batch 1;
            # free: [0, s) = Q^T, [s, 2s) = K^T.
            qkt = pp.tile([128, 2 * s], FP32R, tag="qkt")
            # V augmented: per batch b, key-block g: cols (b*n_kb+g)*65 ..
            # +64 = V rows, col +64 = 1.0 (denominator column).
            v_sb = pp.tile([128, B * n_kb * 65], FP32R, tag="v_sb")
            v3 = v_sb[:].rearrange("p (g c) -> p g c", c=65)
            nc.vector.memset(v3[:, :, 64:65].bitcast(FP32), 1.0)

            for _rep in range(reps):
                _phases(nc, tc, s, n_tg, n_qg, n_kb, x_d, out_d,
                        ident, wqk_r, wv_r, bqk_t, bv128_t, qkt, v3,
                        phases)
            if os.environ.get("MHA_DEBUG") == "1":
                dq_d = nc.dram_tensor("dbg_qkt", [128, 2 * s], FP32,
                                      kind="ExternalOutput")
                dv_d = nc.dram_tensor("dbg_v", [128, B * n_kb * 65], FP32,
                                      kind="ExternalOutput")
                nc.sync.dma_start(dq_d[:], qkt[:].bitcast(FP32))
                nc.sync.dma_start(dv_d[:],
                                  v3[:].rearrange("p g c -> p (g c)")
                                  .bitcast(FP32))
    nc.compile()
    return nc


def _phases(nc, tc, s, n_tg, n_qg, n_kb, x_d, out_d,
            ident, wqk_r, wv_r, bqk_t, bv128_t, qkt, v3, phases="AB"):
    # ---------------- Phase A ----------------
    if "A" in phases:
        with tc.tile_pool(name="pha_sb", bufs=3) as pa, \
             tc.tile_pool(name="pha_vt_sb", bufs=2) as pvt, \
             tc.tile_pool(name="pha_xt_ps", bufs=4, space="PSUM") as pxt, \
             tc.tile_pool(name="pha_qk_ps", bufs=1, space="PSUM") as pqk, \
             tc.tile_pool(name="pha_v_ps", bufs=1, space="PSUM") as pv_ps:
            for tg in range(n_tg):
                b = tg // (n_tg // B)
                toff = (tg % (n_tg // B)) * TG  # token offset in batch
                rb = 64 * b                      # QKT row base
                xnat = pa.tile([128, 2048], FP32, tag="xnat")
                nc.sync.dma_start(
                    xnat[:].rearrange("p (a d) -> p a d", d=D),
                    x_d[tg * TG:(tg + 1) * TG, :]
                    .rearrange("(a p) d -> p a d", p=128))
                xt = pa.tile([128, 2048], FP32R, tag="xt")
                for c in range(4):
                    xtp = pxt.tile([128, 512], FP32, tag="xtp")
                    for a in range(4):
                        nc.tensor.transpose(
                            xtp[:, a * 128:(a + 1) * 128],
                            xnat[:, a * 512 + c * 128:
                                 a * 512 + (c + 1) * 128],
                            ident[:])
                    if c % 2 == 0:
                        nc.scalar.copy(
                            xt[:, c * 512:(c + 1) * 512], xtp[:])
                    else:
                        nc.vector.tensor_copy(
                            xt[:, c * 512:(c + 1) * 512], xtp[:])
                # packed Q^T|K^T projection for this token group
                psqk = pqk.tile([128, TG], FP32, tag="psqk")
                for c in range(4):
                    nc.tensor.matmul(
                        psqk[:], wqk_r[:, c * 128:(c + 1) * 128],
                        xt[:, c * 512:(c + 1) * 512],
                        start=(c == 0), stop=(c == 3))
                nc.vector.tensor_scalar(
                    out=qkt[rb:rb + 64, toff:toff + TG],
                    in0=psqk[0:64, :],
                    scalar1=bqk_t[0:64, :], scalar2=None,
                    op0=mybir.AluOpType.add)
                nc.vector.tensor_scalar(
                    out=qkt[rb:rb + 64, s + toff:s + toff + TG],
                    in0=psqk[64:128, :],
                    scalar1=bqk_t[64:128, :], scalar2=None,
                    op0=mybir.AluOpType.add)
                # V^T projection, then PE-transpose to natural layout
                psvt = pqk.tile([DK, TG], FP32, tag="psvt")
                for c in range(4):
                    nc.tensor.matmul(
                        psvt[:], wv_r[:, c * 64:(c + 1) * 64],
                        xt[:, c * 512:(c + 1) * 512],
                        start=(c == 0), stop=(c == 3))
                vt_sb = pvt.tile([DK, TG], FP32, tag="vt_sb")
                nc.scalar.copy(vt_sb[:], psvt[:])
                vtr = pv_ps.tile([128, 256], FP32, tag="vtr")
                for a in range(4):
                    nc.tensor.transpose(
                        vtr[:, a * 64:(a + 1) * 64],
                        vt_sb[:, a * 128:(a + 1) * 128],
                        ident[0:DK, 0:DK])
                g0 = b * n_kb + (toff // 128)
                nc.vector.tensor_copy(
                    v3[:, g0:g0 + 4, 0:64],
                    vtr[:].rearrange("p (a m) -> p a m", m=64))

    # ---------------- Phase B ----------------
    if "B" not in phases:
        return
    n_grp = (n_kb + GRP - 1) // GRP
    with tc.tile_pool(name="phb_sb", bufs=2) as pb, \
         tc.tile_pool(name="phb_exp", bufs=3) as pexp, \
         tc.tile_pool(name="phb_sc", bufs=2, space="PSUM") as psc, \
         tc.tile_pool(name="phb_pv", bufs=1, space="PSUM") as ppv, \
         tc.tile_pool(name="phb_tr", bufs=1, space="PSUM") as ptr:
        for qg in range(n_qg):
            qoff = qg * QG
            pv = ppv.tile([65, 2 * QG], FP32, tag="pv")
            for g in range(n_grp):
                kb0 = g * GRP
                glen = min(GRP, n_kb - kb0)
                ps = psc.tile([128, GRP * 512], FP32, tag="ps")
                if CUT < 1:
                    continue
                # column layout: all b0 kb-units first, then all b1 units
                # (>=512 cols apart) so the two concurrently-executing
                # row-tiled QK matmuls of a kb pair never share a PSUM bank
                boff = max(glen, 2) * QG
                for j in range(glen):
                    kb = kb0 + j
                    nc.tensor.matmul(
                        ps[:, j * QG:(j + 1) * QG],
                        qkt[0:64, s + kb * 128:s + (kb + 1) * 128],
                        qkt[0:64, qoff:qoff + QG],
                        start=True, stop=True, tile_position=(0, 0))
                    nc.tensor.matmul(
                        ps[:, boff + j * QG:boff + (j + 1) * QG],
                        qkt[64:128, s + kb * 128:s + (kb + 1) * 128],
                        qkt[64:128, qoff:qoff + QG],
                        start=True, stop=True, tile_position=(64, 0))
                cols = boff + glen * QG
                split = SPLIT3 if glen == 3 else min(SPLIT2, cols)
                if CUT < 2:
                    continue
                p = pexp.tile([128, GRP * 512], FP32R, tag="p")
                if glen == 1:  # gap between halves: exp each half (dev-S)
                    for b in range(B):
                        nc.scalar.activation(
                            p[:, b * boff:b * boff + QG],
                            ps[:, b * boff:b * boff + QG],
                            mybir.ActivationFunctionType.Exp, scale=0.125)
                    split = cols
                else:
                    nc.scalar.activation(
                        p[:, 0:split], ps[:, 0:split],
                        mybir.ActivationFunctionType.Exp, scale=0.125)
                if cols > split:
                    sc = pexp.tile([128, GRP * 512 - SPLIT3], INT32,
                                   tag="sch")
                    nc.vector.tensor_scalar(
                        out=sc[:, 0:cols - split],
                        in0=ps[:, split:cols],
                        scalar1=SCH_A, scalar2=SCH_B,
                        op0=mybir.AluOpType.mult,
                        op1=mybir.AluOpType.add)
                    eng = (nc.gpsimd if EXP_MODE == "dvepool"
                           and not NO_GPSIMD else nc.vector)
                    eng.tensor_copy(
                        p[:, split:cols],
                        sc[:, 0:cols - split].bitcast(FP32))
                if CUT < 3:
                    continue
                for j in range(glen):
                    kb = kb0 + j
                    for b in range(B):
                        u = b * boff + j * QG
                        nc.tensor.matmul(
                            pv[:, b * QG:(b + 1) * QG],
                            v3[:, b * n_kb + kb, :],
                            p[:, u:u + QG],
                            start=(kb == 0 and b == 0),
                            stop=(kb == n_kb - 1),
                            skip_group_check=True)
            # epilogue: drain pv, transpose to natural, normalize, bias,
            # store
            if CUT < 3:
                continue
            aug = pb.tile([65, 2 * QG], FP32, tag="aug")
            nc.vector.tensor_copy(aug[:], pv[:])
            if os.environ.get("MHA_DEBUG") == "1" and qg == 0:
                da_d = nc.dram_tensor("dbg_aug", [65, 2 * QG], FP32,
                                      kind="ExternalOutput")
                nc.sync.dma_start(da_d[:], aug[:])
            if CUT < 4:
                continue
            tr = ptr.tile([128, 4 * 65], FP32, tag="tr")
            for a in range(4):  # a = 2*b + j (128-query block j of batch b)
                nc.tensor.transpose(
                    tr[:, a * 65:(a + 1) * 65],
                    aug[:, a * 128:(a + 1) * 128],
                    ident[0:65, 0:65])
            onat = pb.tile([128, 256], FP32, tag="onat")
            for a in range(4):
                rcp = pb.tile([128, 1], FP32, tag=f"rcp{a}")
                nc.vector.reciprocal(
                    rcp[:], tr[:, a * 65 + 64:a * 65 + 65])
                nc.vector.tensor_scalar(
                    out=onat[:, a * 64:(a + 1) * 64],
                    in0=tr[:, a * 65:a * 65 + 64],
                    scalar1=rcp[:], scalar2=None,
                    op0=mybir.AluOpType.mult)
            ofin = pb.tile([128, 256], FP32, tag="ofin")
            for a in range(4):
                _gp(nc).tensor_tensor(
                    out=ofin[:, a * 64:(a + 1) * 64],
                    in0=onat[:, a * 64:(a + 1) * 64],
                    in1=bv128_t[:],
                    op=mybir.AluOpType.add)
            for b in range(B):
                base = b * s + qoff
                nc.sync.dma_start(
                    out_d[base:base + QG, :]
                    .rearrange("(a p) m -> p a m", p=128),
                    ofin[:, b * 128:(b + 1) * 128]
                    .rearrange("p (a m) -> p a m", m=DK))


_NC_CACHE = {}


def _get_nc(s=S, reps=1, phases="AB"):
    key = (s, reps, phases)
    if key not in _NC_CACHE:
        _NC_CACHE[key] = build_nc(s, reps, phases)
    return _NC_CACHE[key]


def make_in_maps(inputs, s=S):
    x = np.ascontiguousarray(np.asarray(inputs["x"], dtype=np.float32))
    toks = B * s
    x_flat = x.reshape(toks, D)
    Wq = np.asarray(inputs["Wq"], dtype=np.float32)
    Wk = np.asarray(inputs["Wk"], dtype=np.float32)
    Wv = np.asarray(inputs["Wv"], dtype=np.float32)
    bq = np.asarray(inputs["bq"], dtype=np.float32)
    bk = np.asarray(inputs["bk"], dtype=np.float32)
    bv = np.asarray(inputs["bv"], dtype=np.float32)
    identr = np.eye(128, dtype=np.float32)
    in_maps = []
    for h in range(N_CORES):
        in_maps.append({
            "x": x_flat,
            "wq": np.ascontiguousarray(Wq[h]),
            "wk": np.ascontiguousarray(Wk[h]),
            "wv": np.ascontiguousarray(Wv[h]),
            "bqk": np.ascontiguousarray(
                np.concatenate([bq[h], bk[h]]).reshape(128, 1)),
            "bv128": np.ascontiguousarray(np.tile(bv[h], (128, 1))),
            "identr": identr,
        })
    return in_maps


def assemble(results, s=S):
    toks = B * s
    out = np.empty((toks, HEADS * DK), dtype=np.float32)
    for h in range(N_CORES):
        out[:, h * DK:(h + 1) * DK] = results[h]["out"]
    return out.reshape(B, s, HEADS * DK)


def kernel(**inputs):
    nc = _get_nc(S)
    res = run_bass_kernel_spmd(nc, make_in_maps(inputs, S),
                               core_ids=list(range(N_CORES)))
    return assemble(res.results, S)
